# revision 24
# baseline (speedup 1.0000x reference)
"""MobileNetV2-MCU head kernel for Trainium2, data-parallel over 8 NeuronCores.

Strategy
--------
- Pure data parallel: batch 32 -> 8 cores x 4 images.
- Per core, channels live on SBUF partitions; the 4 images occupy the four
  32-partition groups wherever the channel count is <=32 (stem/dw1/pw1 stage
  and the pw2/pw3 block outputs), which lets four matmuls run concurrently in
  disjoint 32x32 PE tiles via tile_position.
- 3x3 convs (stem + depthwise) are expressed as PE matmuls:
  * stem: im2col (K=27) gathered by DMA from a host-padded input.
  * depthwise: 9 accumulating matmuls with diagonal weight blocks, reading
    shifted views of a zero-padded SBUF canvas (SAME padding for free).
- BatchNorm is folded into the conv weights host-side; the per-channel bias
  rides the ScalarE eviction (activation func=Relu + bias), ReLU6's upper
  clamp is one VectorE tensor_scalar_min.
- The residual add is fused into the pw3 eviction via scalar_tensor_tensor.
- GAP sum -> tiny FC (bias folded as K=25th row, 1/3136 folded into weights)
  -> softmax on-device.
"""

import numpy as np

BN_EPS = 1e-3
N_CORES = 8
B_PER_CORE = 4

H0 = 224          # input H/W
HP = 225          # host-padded input H/W (SAME pad for stride-2 3x3: +1 bottom/right)
H1 = 112          # stage-1 spatial
C1 = 114          # stage-1 padded canvas side
N1 = H1 * H1      # 12544
CV1 = C1 * C1     # 12996
H2 = 56           # stage-2 spatial
C2 = 58           # stage-2 padded canvas side
N2 = H2 * H2      # 3136
CV2 = C2 * C2     # 3364

CHUNK_ROWS_1 = 4          # output rows per matmul chunk at 112x112 (448 cols)
CHUNK_ROWS_2 = 8          # output rows per matmul chunk at 56x56 (448 cols)
CW1 = CHUNK_ROWS_1 * H1   # 448
CW2 = CHUNK_ROWS_2 * H2   # 448

_PROGRAM_CACHE = {}


# --------------------------------------------------------------------------
# Host-side parameter folding / layout
# --------------------------------------------------------------------------

def _fold_bn(w, bn):
    """w: [...,co]; returns folded w and per-channel bias."""
    a = bn["scale"] / np.sqrt(bn["var"] + BN_EPS)
    b = bn["offset"] - bn["mean"] * a
    return (w * a).astype(np.float32), b.astype(np.float32)


def _block_rep(mat, rows_used):
    """Place mat [rows_used, M] at partition 32b+r for each of the 4 blocks."""
    out = np.zeros((128, mat.shape[1]), np.float32)
    for b in range(B_PER_CORE):
        out[32 * b:32 * b + rows_used, :] = mat
    return out


def _block_bias(vec, rows_used):
    out = np.zeros((128, 1), np.float32)
    for b in range(B_PER_CORE):
        out[32 * b:32 * b + rows_used, 0] = vec
    return out


def _diag_taps(wf, nch, group_cols):
    """wf: [nch, 9] folded depthwise weights (tap t = 3*dy + dx).

    Returns [128, group_cols*9] where for channel ch = 32g + c (c<32, using
    row 32g+c), column group_cols*t + c holds wf[ch, t].
    """
    out = np.zeros((128, group_cols * 9), np.float32)
    for ch in range(nch):
        g, c = divmod(ch, 32)
        for t in range(9):
            out[32 * g + c, group_cols * t + c] = wf[ch, t]
    return out


def _prep_weights(params):
    p = {k: np.asarray(v, np.float32) if not isinstance(v, dict)
         else {kk: np.asarray(vv, np.float32) for kk, vv in v.items()}
         for k, v in params.items()}

    w = {}
    # stem: [3,3,3,32] -> lhsT [27, 32] (row = 9*dx + 3*dy + c, dx-major so the
    # device can load im2col stripes with one 3-dim DMA per (img, dy, dx))
    ws, bs = _fold_bn(p["stem_w"], p["stem_bn"])
    lhsT = np.transpose(ws, (1, 0, 2, 3)).reshape(27, 32)   # (dx, dy, c) -> co
    w["Wstem"] = _block_rep(lhsT, 27)
    w["Bstem"] = _block_bias(bs, 32)

    # dw1: [3,3,1,32] -> wf [32, 9]
    wd, bd = _fold_bn(p["dw1_w"], p["dw1_bn"])
    wf = wd[:, :, 0, :].reshape(9, 32).T          # [32 ch, 9 taps]
    W = np.zeros((128, 288), np.float32)
    for b in range(B_PER_CORE):
        for c in range(32):
            for t in range(9):
                W[32 * b + c, 32 * t + c] = wf[c, t]
    w["Wdw1"] = W
    w["Bdw1"] = _block_bias(bd, 32)

    # pw1: [1,1,32,16]
    wp, bp = _fold_bn(p["pw1_w"], p["pw1_bn"])
    w["Wpw1"] = _block_rep(wp[0, 0], 32)          # [32,16] at each block
    w["Bpw1"] = _block_bias(bp, 16)

    # exp2: [1,1,16,96]
    we, be = _fold_bn(p["exp2_w"], p["exp2_bn"])
    w["Wexp2"] = _block_rep(we[0, 0], 16)         # [16,96]
    B = np.zeros((128, 1), np.float32)
    B[:96, 0] = be
    w["Bexp2"] = B

    # dw2: [3,3,1,96]
    wd, bd = _fold_bn(p["dw2_w"], p["dw2_bn"])
    wf = wd[:, :, 0, :].reshape(9, 96).T
    w["Wdw2"] = _diag_taps(wf, 96, 32)
    B = np.zeros((128, 1), np.float32)
    B[:96, 0] = bd
    w["Bdw2"] = B

    # pw2: [1,1,96,24]
    wp, bp = _fold_bn(p["pw2_w"], p["pw2_bn"])
    W = np.zeros((128, 24), np.float32)
    W[:96, :] = wp[0, 0]
    w["Wpw2"] = W
    w["Bpw2"] = _block_bias(bp, 24)

    # exp3: [1,1,24,144]
    we, be = _fold_bn(p["exp3_w"], p["exp3_bn"])
    w["Wexp3"] = _block_rep(we[0, 0], 24)         # [24,144]
    Ba = np.zeros((128, 1), np.float32)
    Ba[:, 0] = be[:128]
    w["Bexp3a"] = Ba
    Bb = np.zeros((128, 1), np.float32)
    Bb[:16, 0] = be[128:]
    w["Bexp3b"] = Bb

    # dw3: [3,3,1,144]
    wd, bd = _fold_bn(p["dw3_w"], p["dw3_bn"])
    wf = wd[:, :, 0, :].reshape(9, 144).T
    w["Wdw3"] = _diag_taps(wf[:128], 128, 32)
    Wb = np.zeros((128, 144), np.float32)
    for c in range(16):
        for t in range(9):
            Wb[c, 16 * t + c] = wf[128 + c, t]
    w["Wdw3b"] = Wb
    Ba = np.zeros((128, 1), np.float32)
    Ba[:, 0] = bd[:128]
    w["Bdw3a"] = Ba
    Bb = np.zeros((128, 1), np.float32)
    Bb[32:48, 0] = bd[128:]
    w["Bdw3b"] = Bb

    # pw3: [1,1,144,24]
    wp, bp = _fold_bn(p["pw3_w"], p["pw3_bn"])
    Wa = np.zeros((128, 24), np.float32)
    Wa[:, :] = wp[0, 0][:128]
    w["Wpw3a"] = Wa
    Wb = np.zeros((128, 24), np.float32)
    Wb[32:48, :] = wp[0, 0][128:]
    w["Wpw3b"] = Wb
    w["Bpw3"] = _block_bias(bp, 24)

    # fc: [24,1000] + bias as 25th row; GAP 1/N2 folded in
    W = np.zeros((128, 1000), np.float32)
    for b in range(B_PER_CORE):
        W[32 * b:32 * b + 24, :] = p["fc_w"] / float(N2)
        W[32 * b + 24, :] = p["fc_b"]
    w["Wfc"] = W
    # 1.0 at the FC bias rows (32b+24), used to extend GAP vector with a one
    G = np.zeros((128, 1), np.float32)
    for b in range(B_PER_CORE):
        G[32 * b + 24, 0] = 1.0
    w["GapOne"] = G

    # pack everything into one [128, sum(cols)] array -> single DMA, single
    # completion semaphore lane (avoids >4 sync waits on the first matmuls)
    wall = np.concatenate([w[n] for n in WNAMES], axis=1)
    return np.ascontiguousarray(wall)


WNAMES = {
    "Wstem": 32, "Bstem": 1, "Wdw1": 288, "Bdw1": 1, "Wpw1": 16, "Bpw1": 1,
    "Wexp2": 96, "Bexp2": 1, "Wdw2": 288, "Bdw2": 1, "Wpw2": 24, "Bpw2": 1,
    "Wexp3": 144, "Bexp3a": 1, "Bexp3b": 1,
    "Wdw3": 288, "Wdw3b": 144, "Bdw3a": 1, "Bdw3b": 1,
    "Wpw3a": 24, "Wpw3b": 24, "Bpw3": 1, "Wfc": 1000, "GapOne": 1,
}
WALL_COLS = sum(WNAMES.values())


def _build_im2col(xpad):
    """xpad: [N, 225, 225, 3] -> [N, 27, 12544] with row = 9*dx + 3*dy + c."""
    n = xpad.shape[0]
    out = np.zeros((n, 27, N1), np.float32)
    for dx in range(3):
        for dy in range(3):
            v = xpad[:, dy:dy + 223:2, dx:dx + 223:2, :]      # [N,112,112,3]
            out[:, 9 * dx + 3 * dy:9 * dx + 3 * dy + 3, :] = (
                v.transpose(0, 3, 1, 2).reshape(n, 3, N1))
    return out


# --------------------------------------------------------------------------
# Device program
# --------------------------------------------------------------------------

def build_program():
    import concourse.bass as bass
    import concourse.mybir as mybir
    from concourse import bacc
    from concourse.tile import TileContext

    f32 = mybir.dt.float32
    AF = mybir.ActivationFunctionType
    ALU = mybir.AluOpType

    nc = bacc.Bacc("TRN2", target_bir_lowering=False, debug=False)

    # host-im2col'd stem input (row = 32*img + 9*dx + 3*dy + c) + packed weights
    xim = nc.declare_dram_parameter("xim", [128, N1], f32, isOutput=False)
    wall_d = nc.declare_dram_parameter("wall", [128, WALL_COLS], f32,
                                       isOutput=False)
    y = nc.declare_dram_parameter("y", [B_PER_CORE, 1000], f32, isOutput=True)



    def canvas_borders_zero(eng, t, side, nimg):
        """Zero the border ring of each image sub-canvas of tile t."""
        cv = side * side
        for b in range(nimg):
            o = b * cv
            eng.memset(t[:, o:o + side], 0.0)                        # top row
            eng.memset(t[:, o + (side - 1) * side:o + cv], 0.0)      # bottom row
            # right col of row r + left col of row r+1, pairs are adjacent
            mid = t[:, o + side - 1:o + cv - 1].rearrange(
                "p (r two) -> p r two", two=side)[:, :, 0:2]
            eng.memset(mid, 0.0)

    with TileContext(nc) as tc:
        ctx_w = tc.tile_pool(name="wpool", bufs=1, side="left")
        wp = ctx_w.__enter__()
        wall = wp.tile([128, WALL_COLS], f32, name="wall")
        nc.sync.dma_start(out=wall[:, :], in_=wall_d[:, :])
        W = {}
        off = 0
        for n, c in WNAMES.items():
            W[n] = wall[:, off:off + c]
            off += c

        ctx_ps = tc.tile_pool(name="pspool", bufs=1, space="PSUM")
        pp = ctx_ps.__enter__()

        def ps_tile(shape, tag="ps", bufs=3):
            # allocate a full 2KB bank per partition so the psum zero-region
            # bookkeeping stays bank-aligned; hand back a sliced view
            t = pp.tile([shape[0], 512], f32, name="ps_t", tag=tag, bufs=bufs)
            return t[:, :shape[1]]

        def evict_relu6(ps_ap, bias_ap, out_ap):
            nc.scalar.activation(out_ap, ps_ap, AF.Relu, bias=bias_ap)
            nc.vector.tensor_scalar_min(out_ap, out_ap, 6.0)

        def evict_bias(ps_ap, bias_ap, out_ap):
            nc.scalar.activation(out_ap, ps_ap, AF.Identity, bias=bias_ap)

        # ------------------------------------------------ stem + dw1 stage
        ctx_a = tc.tile_pool(name="im2col", bufs=1, side="left")
        pa = ctx_a.__enter__()
        A = pa.tile([128, N1], f32, name="A")
        nc.sync.dma_start(out=A[:, :], in_=xim[:, :])

        ctx_h1 = tc.tile_pool(name="p_h1", bufs=1, side="right")
        ph1 = ctx_h1.__enter__()
        h1 = ph1.tile([128, CV1], f32, name="h1")
        canvas_borders_zero(nc.vector, h1, C1, 1)

        nchunk1 = H1 // CHUNK_ROWS_1
        for k in range(nchunk1):
            ps = ps_tile([128, CW1])
            for b in range(B_PER_CORE):
                nc.tensor.matmul(
                    ps[32 * b:32 * b + 32, :],
                    W["Wstem"][32 * b:32 * b + 27, :],
                    A[32 * b:32 * b + 27, k * CW1:(k + 1) * CW1],
                    start=True, stop=True, tile_position=(32 * b, 32 * b))
            out = h1[:, (k * CHUNK_ROWS_1 + 1) * C1:(k * CHUNK_ROWS_1 + 1 +
                     CHUNK_ROWS_1) * C1].rearrange(
                "p (r c) -> p r c", c=C1)[:, :, 1:1 + H1]
            evict_relu6(ps[:, :].rearrange("p (r c) -> p r c", c=H1),
                        W["Bstem"][:, 0:1], out)
        ctx_a.__exit__(None, None, None)

        # dw1 (3x3 s1 on 112x112, 32ch/img)
        ctx_h2 = tc.tile_pool(name="p_h2", bufs=1, side="left")
        ph2 = ctx_h2.__enter__()
        h2 = ph2.tile([128, N1], f32, name="h2")
        for k in range(nchunk1):
            ps = ps_tile([128, CW1])
            for b in range(B_PER_CORE):
                for t in range(9):
                    dy, dx = divmod(t, 3)
                    rhs = h1[32 * b:32 * b + 32,
                             (k * CHUNK_ROWS_1 + dy) * C1:
                             (k * CHUNK_ROWS_1 + dy + CHUNK_ROWS_1) * C1].rearrange(
                        "p (r c) -> p r c", c=C1)[:, :, dx:dx + H1]
                    nc.tensor.matmul(
                        ps[32 * b:32 * b + 32, :],
                        W["Wdw1"][32 * b:32 * b + 32, 32 * t:32 * t + 32],
                        rhs, start=(t == 0), stop=(t == 8),
                        tile_position=(32 * b, 32 * b))
            evict_relu6(ps[:, :], W["Bdw1"][:, 0:1],
                        h2[:, k * CW1:(k + 1) * CW1])
        ctx_h1.__exit__(None, None, None)

        # pw1 (32->16) + BN, no relu
        ctx_h3 = tc.tile_pool(name="p_h3", bufs=1, side="right")
        ph3 = ctx_h3.__enter__()
        h3 = ph3.tile([128, N1], f32, name="h3")
        for k in range(nchunk1):
            ps = ps_tile([128, CW1])
            for b in range(B_PER_CORE):
                nc.tensor.matmul(
                    ps[32 * b:32 * b + 16, :],
                    W["Wpw1"][32 * b:32 * b + 32, :],
                    h2[32 * b:32 * b + 32, k * CW1:(k + 1) * CW1],
                    start=True, stop=True, tile_position=(32 * b, 32 * b))
            # evict only the 16 used rows per block
            for b in range(B_PER_CORE):
                evict_bias(ps[32 * b:32 * b + 16, :],
                           W["Bpw1"][32 * b:32 * b + 16, 0:1],
                           h3[32 * b:32 * b + 16, k * CW1:(k + 1) * CW1])
        ctx_h2.__exit__(None, None, None)

        # ------------------------------------- stage 2: exp2 -> dw2, per image
        ctx_h5 = tc.tile_pool(name="p_h5", bufs=1, side="left")
        ph5 = ctx_h5.__enter__()
        ctx_h4 = tc.tile_pool(name="p_h4", bufs=1, side="right")
        ph4 = ctx_h4.__enter__()
        h5 = ph5.tile([128, B_PER_CORE * N2], f32, name="h5")

        nchunk2 = H2 // CHUNK_ROWS_2
        for b in range(B_PER_CORE):
            h4 = ph4.tile([128, CV1], f32, name="h4", tag="h4")
            canvas_borders_zero(nc.vector, h4, C1, 1)
            # exp2: 16 -> 96, relu6, into padded canvas
            for k in range(nchunk1):
                ps = ps_tile([128, CW1])
                for cg in range(3):
                    nc.tensor.matmul(
                        ps[32 * cg:32 * cg + 32, :],
                        W["Wexp2"][32 * b:32 * b + 16, 32 * cg:32 * cg + 32],
                        h3[32 * b:32 * b + 16, k * CW1:(k + 1) * CW1],
                        start=True, stop=True, tile_position=(32 * b, 32 * cg))
                out = h4[0:96, (k * CHUNK_ROWS_1 + 1) * C1:
                         (k * CHUNK_ROWS_1 + 1 + CHUNK_ROWS_1) * C1].rearrange(
                    "p (r c) -> p r c", c=C1)[:, :, 1:1 + H1]
                evict_relu6(ps[0:96, :].rearrange("p (r c) -> p r c", c=H1),
                            W["Bexp2"][0:96, 0:1], out)

            # dw2: 3x3 stride 2, 96ch; input canvas row = 2*oh+dy, col = 2*ow+dx
            # (pad_lo=0 -> canvas index +1 shifts: row 2*oh+dy+1 is wrong;
            #  jax SAME s2: in = 2*oh + dy, canvas = in + 1)
            h4r = h4[0:96, :].rearrange("p (r c) -> p r c", c=C1)
            for k in range(nchunk2):
                ps = ps_tile([128, CW2])
                for cg in range(3):
                    for t in range(9):
                        dy, dx = divmod(t, 3)
                        r0 = 2 * (k * CHUNK_ROWS_2) + dy + 1
                        rhs = h4r[32 * cg:32 * cg + 32,
                                  r0:r0 + 2 * CHUNK_ROWS_2 - 1:2,
                                  dx + 1:dx + 2 * H2:2]
                        nc.tensor.matmul(
                            ps[32 * cg:32 * cg + 32, :],
                            W["Wdw2"][32 * cg:32 * cg + 32, 32 * t:32 * t + 32],
                            rhs, start=(t == 0), stop=(t == 8),
                            tile_position=(32 * cg, 32 * cg))
                evict_relu6(ps[0:96, :], W["Bdw2"][0:96, 0:1],
                            h5[0:96, b * N2 + k * CW2:b * N2 + (k + 1) * CW2])
        ctx_h4.__exit__(None, None, None)
        ctx_h3.__exit__(None, None, None)

        # pw2 (96->24) + BN -> block layout h6 [rows 32b+c, N2]
        ctx_h6 = tc.tile_pool(name="p_h6", bufs=1, side="right")
        ph6 = ctx_h6.__enter__()
        h6 = ph6.tile([128, N2], f32, name="h6")
        for k in range(nchunk2):
            ps = ps_tile([128, CW2])
            for b in range(B_PER_CORE):
                nc.tensor.matmul(
                    ps[32 * b:32 * b + 24, :],
                    W["Wpw2"][0:96, :],
                    h5[0:96, b * N2 + k * CW2:b * N2 + (k + 1) * CW2],
                    start=True, stop=True, tile_position=(0, 32 * b))
            for b in range(B_PER_CORE):
                evict_bias(ps[32 * b:32 * b + 24, :],
                           W["Bpw2"][32 * b:32 * b + 24, 0:1],
                           h6[32 * b:32 * b + 24, k * CW2:(k + 1) * CW2])
        ctx_h5.__exit__(None, None, None)

        # exp3 (24->144) relu6 into canvases h7a [128ch] / h7b [16ch]
        ctx_h9 = tc.tile_pool(name="p_h9", bufs=1, side="left")
        ph9 = ctx_h9.__enter__()
        h9 = ph9.tile([128, N2], f32, name="h9")
        nc.vector.memset(h9[:, :], 0.0)   # unused channel rows feed the GAP reduce
        ctx_h7 = tc.tile_pool(name="p_h7", bufs=1, side="left")
        ph7 = ctx_h7.__enter__()
        h7a = ph7.tile([128, B_PER_CORE * CV2], f32, name="h7a", tag="h7a")
        h7b = ph7.tile([16, B_PER_CORE * CV2], f32, name="h7b", tag="h7b")
        canvas_borders_zero(nc.vector, h7a, C2, B_PER_CORE)
        canvas_borders_zero(nc.vector, h7b, C2, B_PER_CORE)
        for b in range(B_PER_CORE):
            for k in range(nchunk2):
                psa = ps_tile([128, CW2])
                psb = ps_tile([32, CW2], tag="psb", bufs=2)
                rhs = h6[32 * b:32 * b + 24, k * CW2:(k + 1) * CW2]
                for cg in range(4):
                    nc.tensor.matmul(
                        psa[32 * cg:32 * cg + 32, :],
                        W["Wexp3"][32 * b:32 * b + 24, 32 * cg:32 * cg + 32],
                        rhs, start=True, stop=True,
                        tile_position=(32 * b, 32 * cg))
                nc.tensor.matmul(
                    psb[0:16, :],
                    W["Wexp3"][32 * b:32 * b + 24, 128:144],
                    rhs, start=True, stop=True, tile_position=(32 * b, 0))
                oa = h7a[:, b * CV2 + (k * CHUNK_ROWS_2 + 1) * C2:
                         b * CV2 + (k * CHUNK_ROWS_2 + 1 + CHUNK_ROWS_2) * C2
                         ].rearrange("p (r c) -> p r c", c=C2)[:, :, 1:1 + H2]
                evict_relu6(psa[:, :].rearrange("p (r c) -> p r c", c=H2),
                            W["Bexp3a"][:, 0:1], oa)
                ob = h7b[0:16, b * CV2 + (k * CHUNK_ROWS_2 + 1) * C2:
                         b * CV2 + (k * CHUNK_ROWS_2 + 1 + CHUNK_ROWS_2) * C2
                         ].rearrange("p (r c) -> p r c", c=C2)[:, :, 1:1 + H2]
                evict_relu6(psb[0:16, :].rearrange("p (r c) -> p r c", c=H2),
                            W["Bexp3b"][0:16, 0:1], ob)

        # dw3 (3x3 s1, 144ch) + pw3 + residual, per image
        ctx_h8 = tc.tile_pool(name="p_h8", bufs=1, side="right")
        ph8 = ctx_h8.__enter__()
        for b in range(B_PER_CORE):
            h8a = ph8.tile([128, N2], f32, name="h8a", tag="h8a")
            h8b = ph8.tile([48, N2], f32, name="h8b", tag="h8b")
            h7ar = h7a[:, b * CV2:(b + 1) * CV2].rearrange(
                "p (r c) -> p r c", c=C2)
            h7br = h7b[0:16, b * CV2:(b + 1) * CV2].rearrange(
                "p (r c) -> p r c", c=C2)
            for k in range(nchunk2):
                psa = ps_tile([128, CW2])
                psb = ps_tile([64, CW2], tag="psb", bufs=2)
                for g in range(4):
                    for t in range(9):
                        dy, dx = divmod(t, 3)
                        r0 = k * CHUNK_ROWS_2 + dy
                        nc.tensor.matmul(
                            psa[32 * g:32 * g + 32, :],
                            W["Wdw3"][32 * g:32 * g + 32, 32 * t:32 * t + 32],
                            h7ar[32 * g:32 * g + 32, r0:r0 + CHUNK_ROWS_2,
                                 dx:dx + H2],
                            start=(t == 0), stop=(t == 8),
                            tile_position=(32 * g, 32 * g))
                for t in range(9):
                    dy, dx = divmod(t, 3)
                    r0 = k * CHUNK_ROWS_2 + dy
                    nc.tensor.matmul(
                        psb[32:48, :],
                        W["Wdw3b"][0:16, 16 * t:16 * t + 16],
                        h7br[0:16, r0:r0 + CHUNK_ROWS_2, dx:dx + H2],
                        start=(t == 0), stop=(t == 8),
                        tile_position=(0, 32))
                evict_relu6(psa[:, :], W["Bdw3a"][:, 0:1],
                            h8a[:, k * CW2:(k + 1) * CW2])
                evict_relu6(psb[32:48, :], W["Bdw3b"][32:48, 0:1],
                            h8b[32:48, k * CW2:(k + 1) * CW2])
            # pw3 (144->24) + bias + residual into h9 block rows
            for k in range(nchunk2):
                ps = ps_tile([128, CW2])
                nc.tensor.matmul(
                    ps[32 * b:32 * b + 24, :],
                    W["Wpw3a"][0:128, :],
                    h8a[:, k * CW2:(k + 1) * CW2],
                    start=True, stop=False, tile_position=(0, 32 * b))
                nc.tensor.matmul(
                    ps[32 * b:32 * b + 24, :],
                    W["Wpw3b"][32:48, :],
                    h8b[32:48, k * CW2:(k + 1) * CW2],
                    start=False, stop=True, tile_position=(32, 32 * b))
                nc.vector.scalar_tensor_tensor(
                    out=h9[32 * b:32 * b + 24, k * CW2:(k + 1) * CW2],
                    in0=ps[32 * b:32 * b + 24, :],
                    scalar=W["Bpw3"][32 * b:32 * b + 24, 0:1],
                    in1=h6[32 * b:32 * b + 24, k * CW2:(k + 1) * CW2],
                    op0=ALU.add, op1=ALU.add)
        ctx_h8.__exit__(None, None, None)
        ctx_h7.__exit__(None, None, None)
        ctx_h6.__exit__(None, None, None)

        # GAP -> FC -> softmax
        ctx_t = tc.tile_pool(name="p_tail", bufs=1, side="right")
        pt = ctx_t.__enter__()
        gap = pt.tile([128, 1], f32, name="gap")
        nc.vector.reduce_sum(gap[:, 0:1], h9[:, :], axis=mybir.AxisListType.X)
        # h9's unused rows reduce to 0; add 1.0 at rows 32b+24 (FC bias row)
        nc.vector.tensor_add(gap[:, 0:1], gap[:, 0:1], W["GapOne"][:, 0:1])

        psfc = pp.tile([128, 1024], f32, name="psfc", tag="psfc", bufs=1)
        for b in range(B_PER_CORE):
            for j, (o, n) in enumerate(((0, 512), (512, 488))):
                nc.tensor.matmul(
                    psfc[32 * b:32 * b + 1, o:o + n],
                    gap[32 * b:32 * b + 25, 0:1],
                    W["Wfc"][32 * b:32 * b + 25, o:o + n],
                    start=True, stop=True, tile_position=(32 * b, 32 * b))

        mx = pt.tile([128, 1], f32, name="mx")
        ssum = pt.tile([128, 1], f32, name="ssum")
        rsum = pt.tile([128, 1], f32, name="rsum")
        ysb = pt.tile([128, 1000], f32, name="ysb")
        for b in range(B_PER_CORE):
            r = slice(32 * b, 32 * b + 1)
            nc.vector.tensor_reduce(mx[r, 0:1], psfc[r, 0:1000],
                                    axis=mybir.AxisListType.X,
                                    op=ALU.max, negate=True)
            nc.scalar.activation(ysb[r, :], psfc[r, 0:1000], AF.Exp,
                                 bias=mx[r, 0:1])
            nc.vector.reduce_sum(ssum[r, 0:1], ysb[r, :],
                                 axis=mybir.AxisListType.X)
            nc.vector.reciprocal(rsum[r, 0:1], ssum[r, 0:1])
            nc.vector.tensor_scalar_mul(ysb[r, :], ysb[r, :], rsum[r, 0:1])

        for b in range(B_PER_CORE):
            nc.sync.dma_start(out=y[b:b + 1, :], in_=ysb[32 * b:32 * b + 1, :])

        ctx_t.__exit__(None, None, None)
        ctx_h9.__exit__(None, None, None)
        ctx_ps.__exit__(None, None, None)
        ctx_w.__exit__(None, None, None)

    nc.compile()
    return nc


def get_program():
    if "nc" not in _PROGRAM_CACHE:
        _PROGRAM_CACHE["nc"] = build_program()
    return _PROGRAM_CACHE["nc"]


# --------------------------------------------------------------------------
# Entry point
# --------------------------------------------------------------------------

def make_in_maps(x, params):
    """Host prep: pad+shard x, fold weights. Returns list of per-core dicts."""
    x = np.asarray(x, np.float32)
    wall = _prep_weights(params)
    xp = np.zeros((x.shape[0], HP, HP, 3), np.float32)
    xp[:, :H0, :H0, :] = x
    im = _build_im2col(xp)                                      # [N, 27, N1]
    in_maps = []
    for core in range(N_CORES):
        xim = np.zeros((128, N1), np.float32)
        for b in range(B_PER_CORE):
            xim[32 * b:32 * b + 27, :] = im[core * B_PER_CORE + b]
        in_maps.append({"xim": xim, "wall": wall})
    return in_maps


def kernel(x, params):
    from concourse.bass_utils import run_bass_kernel_spmd
    nc = get_program()
    in_maps = make_in_maps(x, params)
    res = run_bass_kernel_spmd(nc, in_maps, list(range(N_CORES)))
    out = np.concatenate([r["y"] for r in res.results], axis=0)
    return out.astype(np.float32)
